# revision 31
# baseline (speedup 1.0000x reference)
"""SE(3) attention block (GNN message passing) on 8 Trainium2 NeuronCores.

Strategy
--------
Edges are sorted by destination node on the host. Nodes are cut into tiles of
(<=128 nodes, <=2048 edges); every tile's edges are padded to exactly 2048
slots (16 blocks of 128 edges). Tiles are distributed contiguously across the
8 cores, so every (node, head) softmax group lives entirely on one core and
inside one tile -> no cross-device collectives at all. The destination-node
query vector for each edge slot is pre-gathered on the host (sorted edges ->
a pure layout transform) and shipped transposed next to k^T.

Per node tile the device kernel:
  1. builds the one-hot edge->local-node matrix [e, n] with is_equal
     compares on DVE (one half via an ACT-widened dense dst map in 2x
     mode, one half straight from broadcast per-block dst scalars),
  2. prodT = kT * qgT elementwise (one whole-tile DVE op, bf16 2x),
  3. per-head scores via 16 head-mask matmuls (N=8) into one PSUM bank,
  4. one exp over the tile's [128, 128] scores (ACT, bf16 out),
  5. widens ex to the interleaved 17-stride (ACT) and forms the
     [ex | ex*v] scatter rhs with one whole-tile DVE 2x multiply
     (v is sent from the host with a 1.0 column per head: 17 cols/head),
  6. 16 back-to-back scatter-add matmuls (bf16, N=136) accumulate into a
     [128, 136] PSUM tile.
The tile is then normalized by 1/sum(exp) (strided APs pull ssum/agg out
of the interleaved accumulator) and written out. The host scatters
per-tile rows back into the full [N, 32, 4] output. GPSIMD is left idle
on purpose: concurrent GPSIMD tensor ops contend with DVE for SBUF ports
and inflate both by ~50%.
"""

import math
import numpy as np

# ---------------------------------------------------------------- constants
N_CORES = 8
P = 128                 # partitions / nodes per tile / edges per block
F_BLOCKS = 16           # edge blocks per node tile
EPT = F_BLOCKS * P      # edge slots per tile (2048)
T_PC = 50               # node tiles per core (max 400 total; ~395 needed)
H = 8                   # heads
NF = 128                # features per edge (32*4)
HS = NF // H            # head size (16)
HS1 = HS + 1            # interleaved head stride (ex + 16 features)
N_NODES = 50000
E_EDGES = 800000
PAD_DST = 300.0         # local-dst sentinel for padding edge slots
INV_SQRT_NF = 1.0 / math.sqrt(NF)

_CACHE = {}
LAST_RESULTS = None     # BassKernelResults of the most recent run (for test.py)


# ---------------------------------------------------------------- device IR
def build_nc(tpc=T_PC, f_blocks=F_BLOCKS, v_bf16=True):
    """Build the per-core Bass/Tile program (identical on all 8 cores)."""
    from contextlib import ExitStack

    import concourse.bacc as bacc
    import concourse.mybir as mybir
    from concourse.tile import TileContext

    f32 = mybir.dt.float32
    bf16 = mybir.dt.bfloat16
    vdt = bf16 if v_bf16 else f32
    ept = f_blocks * P

    nc = bacc.Bacc("TRN2", target_bir_lowering=False, debug=False)
    kq_d = nc.dram_tensor("kq", [tpc, P, 2 * ept], bf16, kind="ExternalInput")
    vd_d = nc.dram_tensor("vd", [tpc, P, f_blocks * (H * HS1 + 1)], vdt,
                          kind="ExternalInput")
    io_d = nc.dram_tensor("iota", [P, ept], bf16, kind="ExternalInput")
    hm_d = nc.dram_tensor("hm", [P, H], bf16, kind="ExternalInput")
    out_d = nc.dram_tensor("out", [tpc, P, P], f32, kind="ExternalOutput")

    with TileContext(nc, pool_alloc_mode="queue") as tc, ExitStack() as ctx:
        singles = ctx.enter_context(tc.tile_pool(name="singles", bufs=1))
        big = ctx.enter_context(tc.tile_pool(name="big", bufs=5))
        med = ctx.enter_context(tc.tile_pool(name="med", bufs=3))
        sml = ctx.enter_context(tc.tile_pool(name="sml", bufs=4))
        ps_sc = ctx.enter_context(tc.tile_pool(name="ps_sc", bufs=5, space="PSUM"))
        ps_ag = ctx.enter_context(tc.tile_pool(name="ps_ag", bufs=3, space="PSUM"))

        iota_wide_sb = singles.tile([P, ept], bf16)
        nc.sync.dma_start(out=iota_wide_sb[:], in_=io_d[:, :])
        hm_sb = singles.tile([P, H], bf16)
        nc.sync.dma_start(out=hm_sb[:], in_=hm_d[:, :])

        for t in range(tpc):
            kq_sb = big.tile([P, 2 * ept], bf16, tag="kq")
            nc.sync.dma_start(out=kq_sb[:], in_=kq_d[t])
            kt_sb = kq_sb[:, 0:ept]
            qg_sb = kq_sb[:, ept:2 * ept]
            vd_sb = big.tile([P, f_blocks * (H * HS1 + 1)], vdt, tag="vd")
            nc.sync.dma_start(out=vd_sb[:], in_=vd_d[t])
            v_sb = vd_sb[:, 0:f_blocks * H * HS1]
            dl_sb = vd_sb[:, f_blocks * H * HS1:]

            agg_ps = ps_ag.tile([P, H * HS1], f32, tag="agg")

            hb = f_blocks // 2
            oh_en = med.tile([P, ept], bf16, tag="oh_en", bufs=4)
            prodT = med.tile([P, ept], bf16, tag="prodT", bufs=3)
            c0 = slice(0, hb * P)
            c1 = slice(hb * P, ept)
            # half 0: ACT-widened dst map + 2x DVE compare
            dlw = med.tile([P, hb * P], bf16, tag="dlw", bufs=2)
            nc.scalar.copy(
                out=dlw[:].rearrange("p (b n) -> p b n", b=hb),
                in_=dl_sb[:, 0:hb].to_broadcast([P, hb, P]),
            )
            nc.vector.tensor_tensor(
                out=oh_en[:, c0], in0=iota_wide_sb[:, c0], in1=dlw[:],
                op=mybir.AluOpType.is_equal,
            )
            # half 1: direct broadcast compare on DVE (1x)
            nc.vector.tensor_tensor(
                out=oh_en[:, c1].rearrange("p (b n) -> p b n", b=hb),
                in0=iota_wide_sb[:, c1].rearrange("p (b n) -> p b n", b=hb),
                in1=dl_sb[:, hb:f_blocks].to_broadcast([P, hb, P]),
                op=mybir.AluOpType.is_equal,
            )
            # prodT[f, e] = kT * qgT (DVE, bf16 2x)
            nc.vector.tensor_tensor(
                out=prodT[:], in0=kt_sb[:], in1=qg_sb[:],
                op=mybir.AluOpType.mult,
            )
            # per-head scores for all 16 blocks into one PSUM bank
            sc_ps = ps_sc.tile([P, f_blocks * H], f32, tag="sc")
            for b in range(f_blocks):
                nc.tensor.matmul(
                    out=sc_ps[:, b * H:(b + 1) * H],
                    lhsT=prodT[:, b * P:(b + 1) * P], rhs=hm_sb[:],
                    start=True, stop=True,
                )
            # ex = exp(score / sqrt(nf)) for the whole tile (one ACT op)
            ex_t = sml.tile([P, f_blocks * H], bf16, tag="ex")
            nc.scalar.activation(
                out=ex_t[:], in_=sc_ps[:],
                func=mybir.ActivationFunctionType.Exp,
                scale=INV_SQRT_NF,
            )
            # widen ex to the interleaved stride (one ACT op), then one
            # whole-tile DVE 2x multiply for [ex | ex*v]
            ex_w = med.tile([P, f_blocks * H * HS1], bf16, tag="ex_w", bufs=3)
            evex = med.tile([P, f_blocks * H * HS1], bf16, tag="evex", bufs=3)
            hw = f_blocks * H * HS1 // 2
            for h2 in range(2):
                wcols = slice(h2 * hw, (h2 + 1) * hw)
                nc.scalar.copy(
                    out=ex_w[:, wcols].rearrange("p (x s) -> p x s", s=HS1),
                    in_=ex_t[:, h2 * f_blocks * H // 2:
                             (h2 + 1) * f_blocks * H // 2].to_broadcast(
                        [P, f_blocks * H // 2, HS1]),
                )
            nc.vector.tensor_tensor(
                out=evex[:], in0=v_sb[:], in1=ex_w[:],
                op=mybir.AluOpType.mult,
            )
            # scatter-add all 16 blocks back-to-back (dense PE burst)
            for b in range(f_blocks):
                nc.tensor.matmul(
                    out=agg_ps[:],
                    lhsT=oh_en[:, b * P:(b + 1) * P],
                    rhs=evex[:, b * H * HS1:(b + 1) * H * HS1],
                    start=(b == 0), stop=(b == f_blocks - 1),
                )
            # normalize: out[n, f] = agg[n, f] / ssum[n, h(f)]
            agg_v = agg_ps[:].rearrange("p (h s) -> p h s", h=H)
            inv = sml.tile([P, H], f32, tag="inv")
            nc.vector.tensor_scalar(
                out=inv[:], in0=agg_v[:, :, 0],
                scalar1=1e-30, scalar2=None, op0=mybir.AluOpType.add,
            )
            nc.vector.reciprocal(out=inv[:], in_=inv[:])
            out_sb = med.tile([P, P], f32, tag="out")
            nc.vector.tensor_tensor(
                out=out_sb[:].rearrange("p (h j) -> p h j", h=H),
                in0=agg_v[:, :, 1:],
                in1=inv[:].to_broadcast([P, H, HS]),
                op=mybir.AluOpType.mult,
            )
            nc.sync.dma_start(out=out_d[t], in_=out_sb[:])
    nc.compile()
    return nc


# ------------------------------------------------------------ host plumbing
def _build_tiles(cum, n_nodes, ept):
    """Greedy cut of nodes into (<=128 nodes, <=ept edges) tiles."""
    tiles = []
    n0 = 0
    while n0 < n_nodes:
        n1 = int(np.searchsorted(cum, cum[n0] + ept, side="right")) - 1
        n1 = min(n1, n0 + P, n_nodes)
        if n1 <= n0:
            raise ValueError(f"node {n0} has degree > {ept}; unsupported")
        tiles.append((n0, n1))
        n0 = n1
    return tiles


def _prep_inputs(value, key, query_0, query_1, edge_index,
                 tpc=T_PC, f_blocks=F_BLOCKS, n_cores=N_CORES, v_bf16=True):
    """Sort/tile/pad on the host; returns per-core input maps + assembly info."""
    import ml_dtypes
    bf16 = ml_dtypes.bfloat16

    ept = f_blocks * P
    value = np.ascontiguousarray(np.asarray(value, dtype=np.float32))
    key = np.ascontiguousarray(np.asarray(key, dtype=np.float32))
    q0 = np.asarray(query_0, dtype=np.float32)
    q1 = np.asarray(query_1, dtype=np.float32)
    ei = np.asarray(edge_index)
    n_nodes = q0.shape[0]
    n_edges = key.shape[0]

    dst = ei[1].astype(np.int64).ravel()
    order = np.argsort(dst, kind="stable")
    dsts = dst[order]
    counts = np.bincount(dsts, minlength=n_nodes)
    cum = np.zeros(n_nodes + 1, np.int64)
    cum[1:] = np.cumsum(counts)

    tiles = _build_tiles(cum, n_nodes, ept)
    t_total = len(tiles)
    if t_total > n_cores * tpc:
        raise ValueError(f"{t_total} tiles > capacity {n_cores * tpc}")
    q_per_core = (t_total + n_cores - 1) // n_cores  # real tiles per core
    t8 = n_cores * tpc

    # slot -> original edge id (or padding), per global tile slot
    slot_edge = np.full((t8, ept), 0, np.int64)
    slot_valid = np.zeros((t8, ept), bool)
    slot_dst = np.full((t8, ept), 0, np.int64)   # global dst per slot
    dl = np.full((t8, ept), PAD_DST, np.float32)
    tile_info = []  # (global_tile_idx, n0, n_cnt)
    for i, (n0, n1) in enumerate(tiles):
        c, j = divmod(i, q_per_core)
        idx = c * tpc + j
        e0, e1 = int(cum[n0]), int(cum[n1])
        cnt = e1 - e0
        slot_edge[idx, :cnt] = order[e0:e1]
        slot_valid[idx, :cnt] = True
        slot_dst[idx, :cnt] = dsts[e0:e1]
        dl[idx, :cnt] = (dsts[e0:e1] - n0).astype(np.float32)
        tile_info.append((idx, n0, n1 - n0))

    flat_edge = slot_edge.reshape(-1)
    flat_valid = slot_valid.reshape(-1)

    kf = key.reshape(n_edges, NF)
    k_slots = kf[flat_edge]
    k_slots[~flat_valid] = 0.0
    q_cat = np.concatenate([q0, q1], axis=-1).reshape(
        n_nodes, NF).astype(np.float32)
    qg_slots = q_cat[slot_dst.reshape(-1)]
    qg_slots[~flat_valid] = 0.0
    # merged [kT | qgT]: [t, f, b*128+e] twice, bf16
    kq = np.empty((t8, NF, 2 * ept), bf16)
    kq[:, :, :ept] = k_slots.reshape(
        t8, f_blocks, P, NF).transpose(0, 3, 1, 2).reshape(t8, NF, ept)
    kq[:, :, ept:] = qg_slots.reshape(
        t8, f_blocks, P, NF).transpose(0, 3, 1, 2).reshape(t8, NF, ept)
    del k_slots, qg_slots

    vf = value.reshape(n_edges, NF)
    v_slots = vf[flat_edge]
    v_slots[~flat_valid] = 0.0
    # interleaved v17: [t, e, b, h, 1+16] with a leading 1.0 per head,
    # then the per-block local-dst columns appended: [t, e, b]
    v17 = np.empty((t8, f_blocks, P, H, HS1), np.float32)
    v17[..., 0] = 1.0
    v17[..., 1:] = v_slots.reshape(t8, f_blocks, P, H, HS)
    del v_slots
    vd = np.empty((t8, P, f_blocks * (H * HS1 + 1)), np.float32)
    vd[:, :, :f_blocks * H * HS1] = v17.transpose(0, 2, 1, 3, 4).reshape(
        t8, P, f_blocks * H * HS1)
    del v17
    vd[:, :, f_blocks * H * HS1:] = dl.reshape(
        t8, f_blocks, P).transpose(0, 2, 1)
    vd = vd.astype(bf16) if v_bf16 else vd

    iota = np.broadcast_to(np.arange(P, dtype=np.float32)[None, None, :],
                           (P, F_BLOCKS, P)).reshape(P, F_BLOCKS * P).astype(bf16)
    hm = np.zeros((NF, H), np.float32)
    for h in range(H):
        hm[h * HS:(h + 1) * HS, h] = 1.0
    hm = hm.astype(bf16)

    in_maps = []
    for c in range(n_cores):
        s = slice(c * tpc, (c + 1) * tpc)
        in_maps.append({
            "kq": kq[s], "vd": vd[s], "iota": iota, "hm": hm,
        })
    return in_maps, tile_info, n_nodes


def _assemble(results, tile_info, n_nodes, tpc=T_PC):
    out = np.zeros((n_nodes, NF), np.float32)
    for idx, n0, cnt in tile_info:
        c, j = divmod(idx, tpc)
        out[n0:n0 + cnt] = results[c]["out"][j, :cnt]
    return out.reshape(n_nodes, NF // 4, 4)


def _get_nc(tpc=T_PC, f_blocks=F_BLOCKS, v_bf16=True):
    key = (tpc, f_blocks, v_bf16)
    if key not in _CACHE:
        _CACHE[key] = build_nc(tpc, f_blocks, v_bf16)
    return _CACHE[key]


def _needed_tpc(edge_index, n_nodes, ept, n_cores=N_CORES):
    dst = np.asarray(edge_index)[1].astype(np.int64).ravel()
    counts = np.bincount(dst, minlength=n_nodes)
    cum = np.zeros(n_nodes + 1, np.int64)
    cum[1:] = np.cumsum(counts)
    t_total = len(_build_tiles(cum, n_nodes, ept))
    return (t_total + n_cores - 1) // n_cores


def _run(inputs, trace=False, tpc=T_PC, f_blocks=F_BLOCKS, v_bf16=True,
         **spmd_kwargs):
    global LAST_RESULTS
    from concourse.bass_utils import run_bass_kernel_spmd

    tpc = max(tpc, _needed_tpc(inputs["edge_index"],
                               np.asarray(inputs["query_0"]).shape[0],
                               f_blocks * P))
    nc = _get_nc(tpc, f_blocks, v_bf16)
    in_maps, tile_info, n_nodes = _prep_inputs(
        inputs["value"], inputs["key"], inputs["query_0"], inputs["query_1"],
        inputs["edge_index"], tpc=tpc, f_blocks=f_blocks, v_bf16=v_bf16)
    res = run_bass_kernel_spmd(
        nc, in_maps, list(range(N_CORES)), trace=trace, **spmd_kwargs)
    LAST_RESULTS = res
    return _assemble(res.results, tile_info, n_nodes, tpc=tpc)


def kernel(value, key, query_0, query_1, edge_index):
    return _run({
        "value": value, "key": key, "query_0": query_0,
        "query_1": query_1, "edge_index": edge_index,
    })


# revision 32
# speedup vs baseline: 1.0837x; 1.0837x over previous
"""SE(3) attention block (GNN message passing) on 8 Trainium2 NeuronCores.

Strategy
--------
Edges are sorted by destination node on the host. Nodes are cut into tiles of
(<=128 nodes, <=2048 edges); every tile's edges are padded to exactly 2048
slots (16 blocks of 128 edges). Tiles are distributed contiguously across the
8 cores, so every (node, head) softmax group lives entirely on one core and
inside one tile -> no cross-device collectives at all. The destination-node
query vector for each edge slot is pre-gathered on the host (sorted edges ->
a pure layout transform) and shipped transposed next to k^T.

Per node tile the device kernel:
  1. builds the one-hot edge->local-node matrix [e, n] with is_equal
     compares on DVE (one half via an ACT-widened dense dst map in 2x
     mode, one half straight from broadcast per-block dst scalars),
  2. prodT = kT * qgT elementwise (one whole-tile DVE op, bf16 2x),
  3. per-head scores via 16 head-mask matmuls (N=8) into one PSUM bank,
  4. one exp over the tile's [128, 128] scores (ACT, bf16 out),
  5. widens ex to the interleaved 17-stride (ACT) and forms the
     [ex | ex*v] scatter rhs with one whole-tile DVE 2x multiply
     (v is sent from the host with a 1.0 column per head: 17 cols/head),
  6. 16 back-to-back scatter-add matmuls (bf16, N=136) accumulate into a
     [128, 136] PSUM tile.
The tile is then normalized by 1/sum(exp) (strided APs pull ssum/agg out
of the interleaved accumulator) and written out. The host scatters
per-tile rows back into the full [N, 32, 4] output. GPSIMD is left idle
on purpose: concurrent GPSIMD tensor ops contend with DVE for SBUF ports
and inflate both by ~50%.
"""

import math
import numpy as np

# ---------------------------------------------------------------- constants
N_CORES = 8
P = 128                 # partitions / nodes per tile / edges per block
F_BLOCKS = 16           # edge blocks per node tile
EPT = F_BLOCKS * P      # edge slots per tile (2048)
T_PC = 50               # node tiles per core (max 400 total; ~395 needed)
H = 8                   # heads
NF = 128                # features per edge (32*4)
HS = NF // H            # head size (16)
HS1 = HS + 1            # interleaved head stride (ex + 16 features)
N_NODES = 50000
E_EDGES = 800000
PAD_DST = 300.0         # local-dst sentinel for padding edge slots
INV_SQRT_NF = 1.0 / math.sqrt(NF)

_CACHE = {}
LAST_RESULTS = None     # BassKernelResults of the most recent run (for test.py)


# ---------------------------------------------------------------- device IR
def build_nc(tpc=T_PC, f_blocks=F_BLOCKS, v_bf16=True):
    """Build the per-core Bass/Tile program (identical on all 8 cores)."""
    from contextlib import ExitStack

    import concourse.bacc as bacc
    import concourse.mybir as mybir
    from concourse.tile import TileContext

    f32 = mybir.dt.float32
    bf16 = mybir.dt.bfloat16
    vdt = bf16 if v_bf16 else f32
    ept = f_blocks * P

    nc = bacc.Bacc("TRN2", target_bir_lowering=False, debug=False)
    kq_d = nc.dram_tensor("kq", [tpc, P, 2 * ept], bf16, kind="ExternalInput")
    vd_d = nc.dram_tensor("vd", [tpc, P, f_blocks * (H * HS1 + 1)], vdt,
                          kind="ExternalInput")
    io_d = nc.dram_tensor("iota", [P, ept], bf16, kind="ExternalInput")
    hm_d = nc.dram_tensor("hm", [P, H], bf16, kind="ExternalInput")
    ep_d = nc.dram_tensor("eps", [P, 1], f32, kind="ExternalInput")
    out_d = nc.dram_tensor("out", [tpc, P, P], f32, kind="ExternalOutput")

    with TileContext(nc) as tc, ExitStack() as ctx:
        singles = ctx.enter_context(tc.tile_pool(name="singles", bufs=1))
        big = ctx.enter_context(tc.tile_pool(name="big", bufs=5))
        med = ctx.enter_context(tc.tile_pool(name="med", bufs=3))
        sml = ctx.enter_context(tc.tile_pool(name="sml", bufs=4))
        ps_sc = ctx.enter_context(tc.tile_pool(name="ps_sc", bufs=5, space="PSUM"))
        ps_ag = ctx.enter_context(tc.tile_pool(name="ps_ag", bufs=3, space="PSUM"))

        iota_wide_sb = singles.tile([P, ept], bf16)
        nc.sync.dma_start(out=iota_wide_sb[:], in_=io_d[:, :])
        hm_sb = singles.tile([P, H], bf16)
        nc.sync.dma_start(out=hm_sb[:], in_=hm_d[:, :])
        eps_sb = singles.tile([P, 1], f32)
        nc.sync.dma_start(out=eps_sb[:], in_=ep_d[:, :])

        for t in range(tpc):
            kq_sb = big.tile([P, 2 * ept], bf16, tag="kq")
            nc.sync.dma_start(out=kq_sb[:], in_=kq_d[t])
            kt_sb = kq_sb[:, 0:ept]
            qg_sb = kq_sb[:, ept:2 * ept]
            vd_sb = big.tile([P, f_blocks * (H * HS1 + 1)], vdt, tag="vd")
            nc.sync.dma_start(out=vd_sb[:], in_=vd_d[t])
            v_sb = vd_sb[:, 0:f_blocks * H * HS1]
            dl_sb = vd_sb[:, f_blocks * H * HS1:]

            agg_ps = ps_ag.tile([P, H * HS1], f32, tag="agg")

            hb = f_blocks // 2
            oh_en = med.tile([P, ept], bf16, tag="oh_en", bufs=4)
            prodT = med.tile([P, ept], bf16, tag="prodT", bufs=3)
            c0 = slice(0, hb * P)
            c1 = slice(hb * P, ept)
            # half 0: ACT-widened dst map + 2x DVE compare
            dlw = med.tile([P, hb * P], bf16, tag="dlw", bufs=2)
            nc.scalar.copy(
                out=dlw[:].rearrange("p (b n) -> p b n", b=hb),
                in_=dl_sb[:, 0:hb].to_broadcast([P, hb, P]),
            )
            nc.vector.tensor_tensor(
                out=oh_en[:, c0], in0=iota_wide_sb[:, c0], in1=dlw[:],
                op=mybir.AluOpType.is_equal,
            )
            # half 1: direct broadcast compare on DVE (1x)
            nc.vector.tensor_tensor(
                out=oh_en[:, c1].rearrange("p (b n) -> p b n", b=hb),
                in0=iota_wide_sb[:, c1].rearrange("p (b n) -> p b n", b=hb),
                in1=dl_sb[:, hb:f_blocks].to_broadcast([P, hb, P]),
                op=mybir.AluOpType.is_equal,
            )
            # prodT[f, e] = kT * qgT (DVE, bf16 2x)
            nc.vector.tensor_tensor(
                out=prodT[:], in0=kt_sb[:], in1=qg_sb[:],
                op=mybir.AluOpType.mult,
            )
            # per-head scores for all 16 blocks into one PSUM bank
            sc_ps = ps_sc.tile([P, f_blocks * H], f32, tag="sc")
            for b in range(f_blocks):
                nc.tensor.matmul(
                    out=sc_ps[:, b * H:(b + 1) * H],
                    lhsT=prodT[:, b * P:(b + 1) * P], rhs=hm_sb[:],
                    start=True, stop=True,
                )
            # ex = exp(score / sqrt(nf)) for the whole tile (one ACT op)
            ex_t = sml.tile([P, f_blocks * H], bf16, tag="ex")
            nc.scalar.activation(
                out=ex_t[:], in_=sc_ps[:],
                func=mybir.ActivationFunctionType.Exp,
                scale=INV_SQRT_NF,
            )
            # widen ex to the interleaved stride (one ACT op), then one
            # whole-tile DVE 2x multiply for [ex | ex*v]
            ex_w = med.tile([P, f_blocks * H * HS1], bf16, tag="ex_w", bufs=3)
            evex = med.tile([P, f_blocks * H * HS1], bf16, tag="evex", bufs=3)
            hw = f_blocks * H * HS1 // 2
            for h2 in range(2):
                wcols = slice(h2 * hw, (h2 + 1) * hw)
                nc.scalar.copy(
                    out=ex_w[:, wcols].rearrange("p (x s) -> p x s", s=HS1),
                    in_=ex_t[:, h2 * f_blocks * H // 2:
                             (h2 + 1) * f_blocks * H // 2].to_broadcast(
                        [P, f_blocks * H // 2, HS1]),
                )
            nc.vector.tensor_tensor(
                out=evex[:], in0=v_sb[:], in1=ex_w[:],
                op=mybir.AluOpType.mult,
            )
            # scatter-add all 16 blocks back-to-back (dense PE burst)
            for b in range(f_blocks):
                nc.tensor.matmul(
                    out=agg_ps[:],
                    lhsT=oh_en[:, b * P:(b + 1) * P],
                    rhs=evex[:, b * H * HS1:(b + 1) * H * HS1],
                    start=(b == 0), stop=(b == f_blocks - 1),
                )
            # normalize: out[n, f] = agg[n, f] / ssum[n, h(f)]
            agg_v = agg_ps[:].rearrange("p (h s) -> p h s", h=H)
            agg_sb = sml.tile([P, H * HS1], f32, tag="agg_sb")
            nc.scalar.copy(out=agg_sb[:, H:], in_=agg_v[:, :, 1:])
            inv = sml.tile([P, H], f32, tag="inv")
            nc.scalar.activation(
                out=inv[:], in_=agg_v[:, :, 0],
                func=mybir.ActivationFunctionType.Identity,
                bias=eps_sb[:, 0:1],
            )
            nc.vector.reciprocal(out=inv[:], in_=inv[:])
            out_sb = med.tile([P, P], f32, tag="out")
            nc.gpsimd.tensor_tensor(
                out=out_sb[:].rearrange("p (h j) -> p h j", h=H),
                in0=agg_sb[:, H:].rearrange("p (h j) -> p h j", h=H),
                in1=inv[:].to_broadcast([P, H, HS]),
                op=mybir.AluOpType.mult,
            )
            nc.sync.dma_start(out=out_d[t], in_=out_sb[:])
    nc.compile()
    return nc


# ------------------------------------------------------------ host plumbing
def _build_tiles(cum, n_nodes, ept):
    """Greedy cut of nodes into (<=128 nodes, <=ept edges) tiles."""
    tiles = []
    n0 = 0
    while n0 < n_nodes:
        n1 = int(np.searchsorted(cum, cum[n0] + ept, side="right")) - 1
        n1 = min(n1, n0 + P, n_nodes)
        if n1 <= n0:
            raise ValueError(f"node {n0} has degree > {ept}; unsupported")
        tiles.append((n0, n1))
        n0 = n1
    return tiles


def _prep_inputs(value, key, query_0, query_1, edge_index,
                 tpc=T_PC, f_blocks=F_BLOCKS, n_cores=N_CORES, v_bf16=True):
    """Sort/tile/pad on the host; returns per-core input maps + assembly info."""
    import ml_dtypes
    bf16 = ml_dtypes.bfloat16

    ept = f_blocks * P
    value = np.ascontiguousarray(np.asarray(value, dtype=np.float32))
    key = np.ascontiguousarray(np.asarray(key, dtype=np.float32))
    q0 = np.asarray(query_0, dtype=np.float32)
    q1 = np.asarray(query_1, dtype=np.float32)
    ei = np.asarray(edge_index)
    n_nodes = q0.shape[0]
    n_edges = key.shape[0]

    dst = ei[1].astype(np.int64).ravel()
    order = np.argsort(dst, kind="stable")
    dsts = dst[order]
    counts = np.bincount(dsts, minlength=n_nodes)
    cum = np.zeros(n_nodes + 1, np.int64)
    cum[1:] = np.cumsum(counts)

    tiles = _build_tiles(cum, n_nodes, ept)
    t_total = len(tiles)
    if t_total > n_cores * tpc:
        raise ValueError(f"{t_total} tiles > capacity {n_cores * tpc}")
    q_per_core = (t_total + n_cores - 1) // n_cores  # real tiles per core
    t8 = n_cores * tpc

    # slot -> original edge id (or padding), per global tile slot
    slot_edge = np.full((t8, ept), 0, np.int64)
    slot_valid = np.zeros((t8, ept), bool)
    slot_dst = np.full((t8, ept), 0, np.int64)   # global dst per slot
    dl = np.full((t8, ept), PAD_DST, np.float32)
    tile_info = []  # (global_tile_idx, n0, n_cnt)
    for i, (n0, n1) in enumerate(tiles):
        c, j = divmod(i, q_per_core)
        idx = c * tpc + j
        e0, e1 = int(cum[n0]), int(cum[n1])
        cnt = e1 - e0
        slot_edge[idx, :cnt] = order[e0:e1]
        slot_valid[idx, :cnt] = True
        slot_dst[idx, :cnt] = dsts[e0:e1]
        dl[idx, :cnt] = (dsts[e0:e1] - n0).astype(np.float32)
        tile_info.append((idx, n0, n1 - n0))

    flat_edge = slot_edge.reshape(-1)
    flat_valid = slot_valid.reshape(-1)

    kf = key.reshape(n_edges, NF)
    k_slots = kf[flat_edge]
    k_slots[~flat_valid] = 0.0
    q_cat = np.concatenate([q0, q1], axis=-1).reshape(
        n_nodes, NF).astype(np.float32)
    qg_slots = q_cat[slot_dst.reshape(-1)]
    qg_slots[~flat_valid] = 0.0
    # merged [kT | qgT]: [t, f, b*128+e] twice, bf16
    kq = np.empty((t8, NF, 2 * ept), bf16)
    kq[:, :, :ept] = k_slots.reshape(
        t8, f_blocks, P, NF).transpose(0, 3, 1, 2).reshape(t8, NF, ept)
    kq[:, :, ept:] = qg_slots.reshape(
        t8, f_blocks, P, NF).transpose(0, 3, 1, 2).reshape(t8, NF, ept)
    del k_slots, qg_slots

    vf = value.reshape(n_edges, NF)
    v_slots = vf[flat_edge]
    v_slots[~flat_valid] = 0.0
    # interleaved v17: [t, e, b, h, 1+16] with a leading 1.0 per head,
    # then the per-block local-dst columns appended: [t, e, b]
    v17 = np.empty((t8, f_blocks, P, H, HS1), np.float32)
    v17[..., 0] = 1.0
    v17[..., 1:] = v_slots.reshape(t8, f_blocks, P, H, HS)
    del v_slots
    vd = np.empty((t8, P, f_blocks * (H * HS1 + 1)), np.float32)
    vd[:, :, :f_blocks * H * HS1] = v17.transpose(0, 2, 1, 3, 4).reshape(
        t8, P, f_blocks * H * HS1)
    del v17
    vd[:, :, f_blocks * H * HS1:] = dl.reshape(
        t8, f_blocks, P).transpose(0, 2, 1)
    vd = vd.astype(bf16) if v_bf16 else vd

    iota = np.broadcast_to(np.arange(P, dtype=np.float32)[None, None, :],
                           (P, F_BLOCKS, P)).reshape(P, F_BLOCKS * P).astype(bf16)
    hm = np.zeros((NF, H), np.float32)
    for h in range(H):
        hm[h * HS:(h + 1) * HS, h] = 1.0
    hm = hm.astype(bf16)

    in_maps = []
    for c in range(n_cores):
        s = slice(c * tpc, (c + 1) * tpc)
        in_maps.append({
            "kq": kq[s], "vd": vd[s], "iota": iota, "hm": hm,
            "eps": np.full((P, 1), 1e-30, np.float32),
        })
    return in_maps, tile_info, n_nodes


def _assemble(results, tile_info, n_nodes, tpc=T_PC):
    out = np.zeros((n_nodes, NF), np.float32)
    for idx, n0, cnt in tile_info:
        c, j = divmod(idx, tpc)
        out[n0:n0 + cnt] = results[c]["out"][j, :cnt]
    return out.reshape(n_nodes, NF // 4, 4)


def _get_nc(tpc=T_PC, f_blocks=F_BLOCKS, v_bf16=True):
    key = (tpc, f_blocks, v_bf16)
    if key not in _CACHE:
        _CACHE[key] = build_nc(tpc, f_blocks, v_bf16)
    return _CACHE[key]


def _needed_tpc(edge_index, n_nodes, ept, n_cores=N_CORES):
    dst = np.asarray(edge_index)[1].astype(np.int64).ravel()
    counts = np.bincount(dst, minlength=n_nodes)
    cum = np.zeros(n_nodes + 1, np.int64)
    cum[1:] = np.cumsum(counts)
    t_total = len(_build_tiles(cum, n_nodes, ept))
    return (t_total + n_cores - 1) // n_cores


def _run(inputs, trace=False, tpc=T_PC, f_blocks=F_BLOCKS, v_bf16=True,
         **spmd_kwargs):
    global LAST_RESULTS
    from concourse.bass_utils import run_bass_kernel_spmd

    tpc = max(tpc, _needed_tpc(inputs["edge_index"],
                               np.asarray(inputs["query_0"]).shape[0],
                               f_blocks * P))
    nc = _get_nc(tpc, f_blocks, v_bf16)
    in_maps, tile_info, n_nodes = _prep_inputs(
        inputs["value"], inputs["key"], inputs["query_0"], inputs["query_1"],
        inputs["edge_index"], tpc=tpc, f_blocks=f_blocks, v_bf16=v_bf16)
    res = run_bass_kernel_spmd(
        nc, in_maps, list(range(N_CORES)), trace=trace, **spmd_kwargs)
    LAST_RESULTS = res
    return _assemble(res.results, tile_info, n_nodes, tpc=tpc)


def kernel(value, key, query_0, query_1, edge_index):
    return _run({
        "value": value, "key": key, "query_0": query_0,
        "query_1": query_1, "edge_index": edge_index,
    })
